# revision 1
# baseline (speedup 1.0000x reference)
"""APPNP GNN kernel for 8 TRN2 NeuronCores.

Strategy (node-partitioned, matmul segment-sum):
  - nodes sharded 12500/core (padded 12544 = 128p x 98b, node n -> (p=n//98, b=n%98))
  - MLP prefix data-parallel on TensorE
  - propagation: y = dis*x recursion  y' = c2*(A@y + y) + hh
      per step: AllGather y (bf16, feat padded to 128) -> dma_gather rows per edge
      -> segment-sum via one-hot selection-matrix matmuls into PSUM
  - edges grouped per (dst-block of 128, src-range of 25088 rows) so gather
    indices fit int16; group sizes common across cores (SPMD graph).
"""
import hashlib
import numpy as np

import sys
for _p in ("/opt/trn_rl_repo",):
    if _p not in sys.path:
        sys.path.insert(0, _p)

from concourse import bass, bacc, tile, mybir
from concourse.bass_utils import run_bass_kernel_spmd

F32 = mybir.dt.float32
BF16 = mybir.dt.bfloat16
I16 = mybir.dt.int16

N = 100000
D = 128          # emb dim
H = 256          # hidden
C = 64           # out channels
CP = 128         # padded channels for gather elem (256B)
NC = 8
LOC = N // NC            # 12500
NBLK = 98
BLK = 128
LPAD = NBLK * BLK        # 12544
NRANGE = 4
RSPAN = 2 * LPAD         # 25088
BATCH = 7                # dst blocks per batch (each block gets its own PSUM bank)
NBATCH = NBLK // BATCH   # 14
KSTEPS = 10
ALPHA = 0.1

# debug switches (for bisection)
DBG_NO_AG = False
DBG_NO_GATHER = False
DBG_NO_PGEN = False
DBG_NO_MM = False

_CACHE = {}


def _prep(ei):
    """Host-side graph preprocessing. Returns (common_meta, per_core_arrays, dis)."""
    src = np.asarray(ei[0], np.int64)
    dst = np.asarray(ei[1], np.int64)
    deg = np.bincount(dst, minlength=N).astype(np.float64) + 1.0
    dis = (1.0 / np.sqrt(deg)).astype(np.float32)

    srcpad = (src // LOC) * LPAD + (src % LOC)   # row in padded y_full
    core_of = dst // LOC

    # per-(core, block, range) counts
    per_core_edges = []
    cnt = np.zeros((NC, NBLK, NRANGE), np.int64)
    for c in range(NC):
        m = core_of == c
        s = srcpad[m]
        dl = dst[m] - c * LOC
        b = dl % NBLK                  # dst block (column)
        dp = dl // NBLK                # dst partition within block
        r = s // RSPAN
        il = s % RSPAN
        np.add.at(cnt[c], (b, r), 1)
        per_core_edges.append((b, r, il, dp))

    CH = np.ceil(cnt.max(axis=0) / BLK).astype(np.int64)   # [NBLK, NRANGE] chunks
    # group ordinal: batch-major, then range, then block-within-batch
    # gid = batch*(NRANGE*BATCH) + r*BATCH + (b % BATCH)
    gid_of = np.zeros((NBLK, NRANGE), np.int64)
    sizes = np.zeros(NBLK * NRANGE, np.int64)
    for b in range(NBLK):
        for r in range(NRANGE):
            g = (b // BATCH) * (NRANGE * BATCH) + r * BATCH + (b % BATCH)
            gid_of[b, r] = g
            sizes[g] = CH[b, r] * BLK
    starts = np.zeros(NBLK * NRANGE + 1, np.int64)
    np.cumsum(sizes, out=starts[1:])
    S = int(starts[-1])
    NCHUNK = S // BLK

    # chunk metadata (common): for each chunk: (half, pcol, start, stop)
    # also call layout: per (batch, r): slot span
    chunk_meta = [None] * NCHUNK
    calls = []          # (batch, r, slot_start, n_slots)
    blk_first_chunk = {}
    blk_last_chunk = {}
    for bat in range(NBATCH):
        for r in range(NRANGE):
            g0 = bat * (NRANGE * BATCH) + r * BATCH
            s0 = int(starts[g0])
            n = int(sizes[g0:g0 + BATCH].sum())
            calls.append((bat, r, s0, n))
            for i in range(BATCH):
                b = bat * BATCH + i
                nch = int(CH[b, r])
                c0 = int(starts[g0 + i]) // BLK
                for j in range(nch):
                    cc = c0 + j
                    chunk_meta[cc] = [i, False, False]
                    if b not in blk_first_chunk:
                        blk_first_chunk[b] = cc
                    blk_last_chunk[b] = cc
    for b in range(NBLK):
        assert b in blk_first_chunk, f"block {b} has no chunks"
        chunk_meta[blk_first_chunk[b]][1] = True
        chunk_meta[blk_last_chunk[b]][2] = True

    # per-core slot arrays
    per_core = []
    for c in range(NC):
        b, r, il, dp = per_core_edges[c]
        g = gid_of[b, r]
        order = np.argsort(g, kind="stable")
        gs = g[order]
        # rank within group
        first_idx = np.searchsorted(gs, np.arange(NBLK * NRANGE))
        rank = np.arange(len(gs)) - first_idx[gs]
        pos = starts[gs] + rank
        idx_arr = np.zeros(S, np.int16)
        dst_arr = np.full(S, -1.0, np.float32)
        idx_arr[pos] = il[order].astype(np.int16)
        dst_arr[pos] = dp[order].astype(np.float32)
        # wrap-16 per call, replicate x8
        wraps = []
        for (_bat, _r, s0, n) in calls:
            wraps.append(np.ascontiguousarray(
                idx_arr[s0:s0 + n].reshape(-1, 16).T))
        idx_sb = np.tile(np.hstack(wraps), (8, 1))          # [128, S//16]
        dst_sb = np.ascontiguousarray(dst_arr.reshape(-1, BLK).T)  # [128, NCHUNK]
        per_core.append((idx_sb, dst_sb))

    meta = dict(S=S, NCHUNK=NCHUNK, calls=calls, chunk_meta=chunk_meta,
                max_call=max(cl[3] for cl in calls))
    return meta, per_core, dis


def _build_graph(meta):
    S, NCHUNK = meta["S"], meta["NCHUNK"]
    calls, chunk_meta = meta["calls"], meta["chunk_meta"]
    MAXC = meta["max_call"] // BLK

    nc = bacc.Bacc("TRN2", target_bir_lowering=False, debug=False, num_devices=NC)

    # --- I/O -----------------------------------------------------------
    embT = nc.dram_tensor("embT", [D, LPAD], F32, kind="ExternalInput")
    W1 = nc.dram_tensor("W1", [D, H], F32, kind="ExternalInput")
    b1t = nc.dram_tensor("b1t", [128, 2], F32, kind="ExternalInput")
    W2s = nc.dram_tensor("W2s", [128, 2, C], F32, kind="ExternalInput")
    b2r = nc.dram_tensor("b2r", [128, C], F32, kind="ExternalInput")
    disl_in = nc.dram_tensor("disl", [128, NBLK], F32, kind="ExternalInput")
    c2l_in = nc.dram_tensor("c2l", [128, NBLK], F32, kind="ExternalInput")
    hdisl_in = nc.dram_tensor("hdisl", [128, NBLK], F32, kind="ExternalInput")
    invdl_in = nc.dram_tensor("invdl", [128, NBLK], F32, kind="ExternalInput")
    gidx_in = nc.dram_tensor("gidx", [128, S // 16], I16, kind="ExternalInput")
    pmat_in = nc.dram_tensor("Pmat", [128, NCHUNK, 128], BF16, kind="ExternalInput")
    out = nc.dram_tensor("out", [128, NBLK, C], F32, kind="ExternalOutput")

    # internal DRAM
    ag_in = nc.dram_tensor("ag_in", [128, NBLK, CP], BF16)
    y_full = nc.dram_tensor("y_full", [NC * LPAD, CP], BF16, addr_space="Shared")

    with tile.TileContext(nc) as tc:
        with tc.tile_pool(name="const", bufs=1) as cp:
            disl = cp.tile([128, NBLK], F32, name="disl_sb")
            c2l = cp.tile([128, NBLK], F32, name="c2l_sb")
            hdisl = cp.tile([128, NBLK], F32, name="hdisl_sb")
            invdl = cp.tile([128, NBLK], F32, name="invdl_sb")
            b2rep = cp.tile([128, C], F32, name="b2r_sb")
            b1sb = cp.tile([128, 2], F32, name="b1_sb")
            idx_sb = cp.tile([128, S // 16], I16, name="idx_sb")
            y = cp.tile([128, NBLK, C], F32, name="y_sb")
            hh = cp.tile([128, NBLK, C], F32, name="hh_sb")
            y_bf = cp.tile([128, NBLK, CP], BF16, name="ybf_sb")

            nc.sync.dma_start(disl[:], disl_in[:])
            nc.sync.dma_start(c2l[:], c2l_in[:])
            nc.sync.dma_start(hdisl[:], hdisl_in[:])
            nc.sync.dma_start(invdl[:], invdl_in[:])
            nc.sync.dma_start(b2rep[:], b2r[:])
            nc.sync.dma_start(b1sb[:], b1t[:])
            nc.sync.dma_start(idx_sb[:], gidx_in[:])
            nc.vector.memset(y_bf[:, :, C:CP], 0.0)

            # ---------------- MLP prefix ------------------------------
            with tc.tile_pool(name="mlp", bufs=2) as mp, \
                 tc.tile_pool(name="mlpp", bufs=2, space="PSUM") as mpp, \
                 tc.tile_pool(name="mlpp2", bufs=4, space="PSUM") as mpp2:
                w1sb = mp.tile([D, H], F32, name="w1_sb", bufs=1)
                w2sb = mp.tile([128, 2, C], F32, name="w2_sb", bufs=1)
                embsb = mp.tile([D, LPAD], F32, name="emb_sb", bufs=1)
                nc.sync.dma_start(w1sb[:], W1[:])
                nc.sync.dma_start(w2sb[:], W2s[:])
                nc.sync.dma_start(embsb[:], embT[:])
                ntile = (LPAD + 511) // 512
                for t in range(ntile):
                    w = min(512, LPAD - t * 512)
                    ps1a = mpp.tile([128, 512], F32, name="ps1a", tag="ps1a")
                    ps1b = mpp.tile([128, 512], F32, name="ps1b", tag="ps1b")
                    nc.tensor.matmul(ps1a[:, :w], w1sb[:, 0:128],
                                     embsb[:, t * 512:t * 512 + w],
                                     start=True, stop=True)
                    nc.tensor.matmul(ps1b[:, :w], w1sb[:, 128:256],
                                     embsb[:, t * 512:t * 512 + w],
                                     start=True, stop=True)
                    h1a = mp.tile([128, 512], F32, name="h1a", tag="h1a")
                    h1b = mp.tile([128, 512], F32, name="h1b", tag="h1b")
                    nc.scalar.activation(h1a[:, :w], ps1a[:, :w],
                                         mybir.ActivationFunctionType.Relu,
                                         bias=b1sb[:, 0:1])
                    nc.scalar.activation(h1b[:, :w], ps1b[:, :w],
                                         mybir.ActivationFunctionType.Relu,
                                         bias=b1sb[:, 1:2])
                    for sub in range(w // 128):
                        blk = t * 4 + sub
                        ps2 = mpp2.tile([128, C], F32, name="ps2", tag="ps2")
                        nc.tensor.matmul(ps2[:], h1a[:, sub * 128:(sub + 1) * 128],
                                         w2sb[:, 0, :], start=True, stop=False)
                        nc.tensor.matmul(ps2[:], h1b[:, sub * 128:(sub + 1) * 128],
                                         w2sb[:, 1, :], start=False, stop=True)
                        x0t = mp.tile([128, C], F32, name="x0t", tag="x0t")
                        nc.vector.tensor_add(x0t[:], ps2[:], b2rep[:])
                        nc.vector.tensor_mul(y[:, blk, :], x0t[:],
                                             disl[:, blk:blk + 1].broadcast_to((128, C)))
                        nc.vector.tensor_mul(hh[:, blk, :], x0t[:],
                                             hdisl[:, blk:blk + 1].broadcast_to((128, C)))
                        nc.scalar.copy(y_bf[:, blk, 0:C], y[:, blk, :])

            # ---------------- propagation steps -----------------------
            with tc.tile_pool(name="gp", bufs=3) as gp, \
                 tc.tile_pool(name="pp", bufs=6) as ppool, \
                 tc.tile_pool(name="tp", bufs=4) as tp, \
                 tc.tile_pool(name="psp", bufs=8, space="PSUM") as psp:
                for k in range(KSTEPS):
                    nc.sync.dma_start(ag_in[:], y_bf[:])
                    if not DBG_NO_AG:
                        nc.gpsimd.collective_compute(
                            "AllGather", mybir.AluOpType.bypass,
                            replica_groups=[list(range(NC))],
                            ins=[ag_in[:].opt()], outs=[y_full[:].opt()])

                    ci = 0  # call index
                    for bat in range(NBATCH):
                        pstiles = [psp.tile([128, C], F32, name="ps", tag="ps")
                                   for _ in range(BATCH)]
                        for r in range(NRANGE):
                            _bat, _r, s0, n = calls[ci]
                            assert _bat == bat and _r == r
                            ci += 1
                            if n == 0:
                                continue
                            g = gp.tile([128, MAXC, CP], BF16, name="g", tag="g")
                            ncol = n // BLK
                            if not DBG_NO_GATHER:
                                nc.gpsimd.dma_gather(
                                    g[:, :ncol, :],
                                    y_full[r * RSPAN:(r + 1) * RSPAN, :],
                                    idx_sb[:, s0 // 16:(s0 + n) // 16],
                                    num_idxs=n, num_idxs_reg=n, elem_size=CP,
                                    single_packet=False)
                            else:
                                nc.gpsimd.memset(g[:, 0, :], 0.0)
                            c0 = s0 // BLK
                            P = ppool.tile([128, MAXC, 128], BF16, name="P", tag="P")
                            nc.sync.dma_start(P[:, :ncol, :],
                                              pmat_in[:, c0:c0 + ncol, :])
                            for j in range(ncol):
                                cc = c0 + j
                                i, st, sp = chunk_meta[cc]
                                if not DBG_NO_MM:
                                    nc.tensor.matmul(pstiles[i][:],
                                                     P[:, j, :], g[:, j, 0:C],
                                                     start=st, stop=sp)
                        for i in range(BATCH):
                            blk = bat * BATCH + i
                            t1 = tp.tile([128, C], F32, name="t1", tag="t1")
                            nc.vector.tensor_add(t1[:], pstiles[i][:],
                                                 y[:, blk, :])
                            nc.vector.tensor_mul(
                                t1[:], t1[:],
                                c2l[:, blk:blk + 1].broadcast_to((128, C)))
                            nc.vector.tensor_add(y[:, blk, :], t1[:],
                                                 hh[:, blk, :])
                            if k < KSTEPS - 1:
                                nc.scalar.copy(y_bf[:, blk, 0:C], y[:, blk, :])

                # final: x = y / dis -> out
                for blk in range(NBLK):
                    nc.vector.tensor_mul(
                        hh[:, blk, :], y[:, blk, :],
                        invdl[:, blk:blk + 1].broadcast_to((128, C)))
                nc.sync.dma_start(out[:], hh[:])

    nc.compile()
    return nc


def _make_inputs(x_indices, ei, emb, W1, b1, W2, b2, meta, per_core, dis):
    import ml_dtypes
    xe = np.asarray(emb, np.float32)[np.asarray(x_indices, np.int64)]
    W1 = np.asarray(W1, np.float32)
    b1 = np.asarray(b1, np.float32)
    W2 = np.asarray(W2, np.float32)
    b2 = np.asarray(b2, np.float32)

    b1t = np.ascontiguousarray(b1.reshape(2, 128).T)         # [128, 2]
    W2s = np.ascontiguousarray(W2.reshape(2, 128, C).transpose(1, 0, 2))
    b2rep = np.broadcast_to(b2, (128, C)).copy()
    dgrid = np.arange(128, dtype=np.float32)[None, None, :]

    in_maps = []
    for c in range(NC):
        # local nodes: node n -> (p = n // NBLK, b = n % NBLK), n in [0, LPAD)
        nloc = np.arange(LPAD)
        gl = c * LOC + nloc
        valid = nloc < LOC
        disv = np.where(valid, dis[np.minimum(gl, N - 1)], 0.0).astype(np.float32)
        grid = disv.reshape(128, NBLK)
        disl = grid
        c2l = ((1.0 - ALPHA) * grid * grid).astype(np.float32)
        hdisl = (ALPHA * grid).astype(np.float32)
        with np.errstate(divide="ignore"):
            invd = np.where(grid > 0, 1.0 / grid, 0.0).astype(np.float32)
        # embT: column col = blk*128 + p  <->  node p*NBLK + blk
        xe_loc = np.zeros((LPAD, D), np.float32)
        xe_loc[:LOC] = xe[c * LOC:(c + 1) * LOC]
        cols = np.arange(LPAD)
        colnode = (cols % 128) * NBLK + (cols // 128)
        embT = np.ascontiguousarray(xe_loc[colnode].T)  # [D, LPAD]
        idx_sb, dst_sb = per_core[c]
        # host-side one-hot selection matrices, slot-major for full-rate DMA:
        # Pmat[p, cc, d] = 1.0 iff dst_sb[p, cc] == d
        pmat = (dst_sb[:, :, None] == dgrid).astype(ml_dtypes.bfloat16)
        in_maps.append({
            "embT": embT, "W1": W1, "b1t": b1t, "W2s": W2s, "b2r": b2rep,
            "disl": disl, "c2l": c2l, "hdisl": hdisl,
            "invdl": invd, "gidx": idx_sb, "Pmat": pmat,
        })
    return in_maps


def kernel(x_indices, ei, emb, W1, b1, W2, b2, _bench=False):
    key = hashlib.sha1(np.ascontiguousarray(ei).tobytes()).hexdigest()
    if key not in _CACHE:
        meta, per_core, dis = _prep(ei)
        nc = _build_graph(meta)
        _CACHE[key] = (meta, per_core, dis, nc)
    meta, per_core, dis, nc = _CACHE[key]
    in_maps = _make_inputs(x_indices, ei, emb, W1, b1, W2, b2,
                           meta, per_core, dis)
    res = run_bass_kernel_spmd(nc, in_maps, core_ids=list(range(NC)),
                               trace=bool(_bench))
    outs = []
    for c in range(NC):
        o = res.results[c]["out"]            # [128, NBLK, C]
        rows = np.asarray(o, np.float32).reshape(LPAD, C)
        outs.append(rows[:LOC])
    full = np.concatenate(outs, axis=0)
    if _bench:
        return full, res
    return full

